# revision 1
# baseline (speedup 1.0000x reference)
"""2-layer GraphSAGE (mean aggregation) on 8 trn2 NeuronCores via Bass/Tile.

Strategy (matches the sharding hint):
  - Nodes are row-sharded across the 8 cores (6250 rows each); edges are
    partitioned by destination core.
  - Per core, edges are grouped by 128-node destination block.  Messages
    x[src] are fetched with InstDMAGatherAnt (edge-major tiles of 128) from a
    bf16 256B-padded copy of the features, and the segment-sum is computed as
    a one-hot matmul on the tensor engine (bf16, 1 cycle/row):
        aggT[64f, 128d] += msgs[128e, 64f].T @ onehot[128e, 128d]
    where onehot[e, d] = (d == dst_local[e]) is built on the vector engine
    from a broadcast iota with one tensor_scalar(is_equal) op per tile.
    The exact f32 1/deg scaling is applied at PSUM->SBUF copy time via a
    host-built [64, n] broadcast table (elementwise mult on DVE).
  - The 64x64 weights are replicated; the dense phase runs feature-major in
    f32 on rotating [64, 512] group buffers.
  - h = tanh(layer1) chunks (bf16-padded) are AllGathered between layers.
  - dma_gather indices are int16, so each gather call reads one of two row
    regions of the source, split at a core boundary so lo/hi membership is
    identical for the x-space and the padded h-space.
"""

import numpy as np
import ml_dtypes

import concourse.bacc as bacc
import concourse.mybir as mybir
import concourse.tile as tile
from concourse.bass_utils import run_bass_kernel_spmd

P = 128
D = 64
F32 = mybir.dt.float32
BF16 = mybir.dt.bfloat16
I16 = mybir.dt.int16
BF = ml_dtypes.bfloat16


class Cfg:
    def __init__(self, N, n_cores=8, chunk=64, msgs_bufs=4):
        assert N % n_cores == 0
        self.N = N
        self.n_cores = n_cores
        self.n_own = N // n_cores
        self.nblk = -(-self.n_own // P)
        self.n_own_pad = self.nblk * P
        self.n_pad_all = self.n_own_pad * n_cores
        # lo/hi split at a core boundary so that edge region membership is
        # identical for x-space (N rows) and padded h-space (n_pad_all rows).
        c = n_cores // 2
        while self.N - c * self.n_own > 32768 or self.n_pad_all - c * self.n_own_pad > 32768:
            c += 1
        assert c * self.n_own <= 32768 and c * self.n_own_pad <= 32768
        self.split_core = c
        self.split = c * self.n_own
        self.split_pad = c * self.n_own_pad
        self.chunk = chunk
        self.msgs_bufs = msgs_bufs


class Meta:
    pass


def _wrap16(v):
    """slot i -> [i % 16, i // 16] layout used by dma_gather idx tables."""
    assert v.shape[0] % 16 == 0
    return np.ascontiguousarray(v.reshape(-1, 16).T)


def preprocess(edge_index, cfg):
    """Partition/group edges; build per-core gather index + onehot tables."""
    src = np.asarray(edge_index[0], dtype=np.int64)
    dst = np.asarray(edge_index[1], dtype=np.int64)
    E = src.shape[0]
    NC, NBLK = cfg.n_cores, cfg.nblk

    cnt = np.bincount(dst, minlength=cfg.N).astype(np.float32)
    inv = (1.0 / np.maximum(cnt, 1.0)).astype(np.float32)

    core = dst // cfg.n_own
    dstl = dst - core * cfg.n_own
    blk = dstl // P
    inb = dstl - blk * P
    region = (src >= cfg.split).astype(np.int64)

    key = ((core * NBLK) + blk) * 2 + region
    ngroups = NC * NBLK * 2
    gcnt = np.bincount(key, minlength=ngroups).reshape(NC, NBLK, 2)
    # uniform (max over cores) tile counts per (block, region)
    TL = np.maximum(1, -(-gcnt[:, :, 0].max(axis=0) // P))
    TH = np.maximum(1, -(-gcnt[:, :, 1].max(axis=0) // P))
    lo_off = np.concatenate([[0], np.cumsum(TL)])
    hi_off = np.concatenate([[0], np.cumsum(TH)])
    TLT, THT = int(lo_off[-1]), int(hi_off[-1])
    T_ALL = TLT + THT

    # rank of each edge within its (core, blk, region) group
    order = np.argsort(key, kind="stable")
    gstart = np.concatenate([[0], np.cumsum(np.bincount(key, minlength=ngroups))])[:-1]
    rank = np.empty(E, dtype=np.int64)
    rank[order] = np.arange(E) - gstart[key[order]]

    # slot within region (tiles of 128)
    reg_base = np.where(region == 0, lo_off[blk], hi_off[blk])
    slot = reg_base * P + rank

    # h-space (padded) position of each source node
    pos = (src // cfg.n_own) * cfg.n_own_pad + (src % cfg.n_own)

    meta = Meta()
    meta.cfg = cfg
    meta.TL, meta.TH = TL, TH
    meta.TLT, meta.THT, meta.T_ALL = TLT, THT, T_ALL
    meta.block_tiles = [
        list(range(int(lo_off[b]), int(lo_off[b + 1])))
        + [TLT + t for t in range(int(hi_off[b]), int(hi_off[b + 1]))]
        for b in range(NBLK)
    ]

    # per-core tables
    meta.idx = []   # [128, 8*(TLT+THT)*2] int16 : l1lo | l1hi | l2lo | l2hi
    meta.dstf = []  # [128, T_ALL] f32
    meta.invb = []  # [64, n_own_pad] f32 : 1/deg broadcast down 64 partitions
    for k in range(NC):
        m = core == k
        sl = slot[m]
        rg = region[m]
        s_lo, s_hi = sl[rg == 0], sl[rg == 1]
        i1lo = np.zeros(TLT * P, np.int16)
        i1hi = np.zeros(THT * P, np.int16)
        i2lo = np.zeros(TLT * P, np.int16)
        i2hi = np.zeros(THT * P, np.int16)
        i1lo[s_lo] = src[m][rg == 0]
        i1hi[s_hi] = src[m][rg == 1] - cfg.split
        i2lo[s_lo] = pos[m][rg == 0]
        i2hi[s_hi] = pos[m][rg == 1] - cfg.split_pad
        w = np.concatenate(
            [_wrap16(a) for a in (i1lo, i1hi, i2lo, i2hi)], axis=1)
        # the gather ucode reads each Q7 core's idx stripe from its own
        # 16-partition group -> replicate 8x down the partition axis
        meta.idx.append(np.ascontiguousarray(np.tile(w, (8, 1))))

        df = np.full(T_ALL * P, -1.0, np.float32)
        gs = np.where(rg == 0, 0, TLT * P) + sl
        df[gs] = inb[m].astype(np.float32)
        meta.dstf.append(np.ascontiguousarray(df.reshape(T_ALL, P).T))

        iv = np.ones(cfg.n_own_pad, np.float32)
        iv[:cfg.n_own] = inv[k * cfg.n_own:(k + 1) * cfg.n_own]
        meta.invb.append(np.ascontiguousarray(np.tile(iv, (D, 1))))

    meta.idx_off = [0, TLT * 8, (TLT + THT) * 8, (2 * TLT + THT) * 8]

    # gather calls: (region, t0, ntiles, first_block), interleaved by the
    # first destination block each chunk serves.
    def chunks(T_total, offs):
        out = []
        t0 = 0
        while t0 < T_total:
            nt = min(cfg.chunk, T_total - t0)
            fb = int(np.searchsorted(offs, t0, side="right") - 1)
            out.append((t0, nt, fb))
            t0 += nt
        return out

    calls = [(0, t0, nt, fb) for (t0, nt, fb) in chunks(TLT, lo_off)]
    calls += [(1, t0, nt, fb) for (t0, nt, fb) in chunks(THT, hi_off)]
    calls.sort(key=lambda c: (c[3], c[0]))
    meta.calls = calls
    return meta


GCOL = 512  # dense-phase group width (one PSUM bank)


def build_program(meta, one_core=False,
                  parts=("gather", "agg", "dense", "store", "collective"),
                  reps=1):
    cfg = meta.cfg
    NC, NBLK = cfg.n_cores, cfg.nblk
    NP = cfg.n_own_pad
    BPG = GCOL // P  # blocks per dense group
    nc = bacc.Bacc(
        "TRN2", target_bir_lowering=False, debug=False,
        num_devices=1 if one_core else NC,
    )

    xp_dr = nc.dram_tensor("xp", [cfg.N, P], BF16, kind="ExternalInput")
    xoT_dr = nc.dram_tensor("xoT", [D, NP], F32, kind="ExternalInput")
    idx_dr = nc.dram_tensor("idx", list(meta.idx[0].shape), I16, kind="ExternalInput")
    dstf_dr = nc.dram_tensor("dstf", [P, meta.T_ALL], F32, kind="ExternalInput")
    invb_dr = nc.dram_tensor("invb", [D, NP], F32, kind="ExternalInput")
    wl1_dr = nc.dram_tensor("wl1t", [D, D], F32, kind="ExternalInput")
    wr1_dr = nc.dram_tensor("wr1t", [D, D], F32, kind="ExternalInput")
    wl2_dr = nc.dram_tensor("wl2t", [D, D], F32, kind="ExternalInput")
    wr2_dr = nc.dram_tensor("wr2t", [D, D], F32, kind="ExternalInput")
    b1_dr = nc.dram_tensor("b1", [D, 1], F32, kind="ExternalInput")
    b2_dr = nc.dram_tensor("b2", [D, 1], F32, kind="ExternalInput")
    iota_dr = nc.dram_tensor("iota", [P, P], BF16, kind="ExternalInput")
    id_dr = nc.dram_tensor("ident", [D, D], F32, kind="ExternalInput")
    out_dr = nc.dram_tensor("out", [NP, D], F32, kind="ExternalOutput")

    with tile.TileContext(nc) as tc:
        with (
            tc.tile_pool(name="const", bufs=1) as cpool,
            tc.tile_pool(name="big", bufs=1) as bpool,
            tc.tile_pool(name="msgs", bufs=cfg.msgs_bufs) as mpool,
            tc.tile_pool(name="idxp", bufs=4) as ipool,
            tc.tile_pool(name="ohp", bufs=12) as ohpool,
            tc.tile_pool(name="grp", bufs=2) as gpool,
            tc.tile_pool(name="psA", bufs=4, space="PSUM") as psA,
            tc.tile_pool(name="psZ", bufs=2, space="PSUM") as psZ,
            tc.tile_pool(name="psT", bufs=2, space="PSUM") as psT,
            tc.tile_pool(name="dram", bufs=1, space="DRAM") as dpool,
        ):
            def load(pool, dr, shape, name, dt=F32, tag=""):
                t = pool.tile(shape, dt, name=name, tag=tag or name)
                nc.sync.dma_start(out=t, in_=dr.ap())
                return t

            iota_sb = load(cpool, iota_dr, [P, P], "iota_sb", dt=BF16)
            ident_sb = load(cpool, id_dr, [D, D], "ident_sb")
            wl1_sb = load(cpool, wl1_dr, [D, D], "wl1_sb")
            wr1_sb = load(cpool, wr1_dr, [D, D], "wr1_sb")
            wl2_sb = load(cpool, wl2_dr, [D, D], "wl2_sb")
            wr2_sb = load(cpool, wr2_dr, [D, D], "wr2_sb")
            b1_sb = load(cpool, b1_dr, [D, 1], "b1_sb")
            b2_sb = load(cpool, b2_dr, [D, 1], "b2_sb")
            dstf_sb = load(bpool, dstf_dr, [P, meta.T_ALL], "dstf_sb")
            invb_sb = load(bpool, invb_dr, [D, NP], "invb_sb")
            xoT_sb = load(bpool, xoT_dr, [D, NP], "xoT_sb")
            hT_sb = bpool.tile([D, NP], F32, name="hT_sb")
            nodeh_sb = bpool.tile([P, NBLK * P], BF16, name="nodeh_sb")
            nodeo_sb = bpool.tile([P, NBLK * D], F32, name="nodeo_sb")
            # zero the bf16 pad columns once (cols [b*128+64, b*128+128))
            nc.vector.memset(nodeh_sb, 0.0)

            for rep in range(reps):
              h_chunk = dpool.tile([NP, P], BF16, name=f"h_chunk_{rep}", tag=f"hc{rep}")
              h_full = dpool.tile([cfg.n_pad_all, P], BF16, name=f"h_full_{rep}",
                                  tag=f"hf{rep}", addr_space="Shared")
              for layer in range(2):
                if layer == 0:
                    src_lo = xp_dr.ap()[0:cfg.split, :]
                    src_hi = xp_dr.ap()[cfg.split:cfg.N, :]
                    off_lo, off_hi = meta.idx_off[0], meta.idx_off[1]
                else:
                    src_lo = h_full[0:cfg.split_pad, :]
                    src_hi = h_full[cfg.split_pad:cfg.n_pad_all, :]
                    off_lo, off_hi = meta.idx_off[2], meta.idx_off[3]

                # ---- gather messages (bf16, 256B rows) ----
                tsrc = {}
                for ci, (rg, t0, ntile, _fb) in enumerate(meta.calls):
                    mt = mpool.tile([P, cfg.chunk, P], BF16, tag="msgs",
                                    name=f"m_{layer}_{ci}")
                    if "gather" in parts:
                        it = ipool.tile([P, cfg.chunk * 8], I16, tag="idx",
                                        name=f"i_{layer}_{ci}")
                        cols = ntile * 8
                        coff = (off_lo if rg == 0 else off_hi) + t0 * 8
                        nc.sync.dma_start(out=it[:, :cols],
                                          in_=idx_dr.ap()[:, coff:coff + cols])
                        nc.gpsimd.dma_gather(
                            mt[:, :ntile, :],
                            src_lo if rg == 0 else src_hi,
                            it[:, :cols],
                            num_idxs=ntile * P,
                            num_idxs_reg=ntile * P,
                            elem_size=P,
                            single_packet=False,
                        )
                    base = t0 if rg == 0 else meta.TLT + t0
                    for j in range(ntile):
                        tsrc[base + j] = (mt, j)

                # ---- blocks: onehot matmul segment-sum + dense per group ----
                if layer == 0:
                    wl_sb, wr_sb, bb_sb = wl1_sb, wr1_sb, b1_sb
                    own_sb = xoT_sb
                    func = mybir.ActivationFunctionType.Tanh
                else:
                    wl_sb, wr_sb, bb_sb = wl2_sb, wr2_sb, b2_sb
                    own_sb = hT_sb
                    func = mybir.ActivationFunctionType.Identity

                ngrp = -(-NBLK // BPG)
                for g in range(ngrp if "agg" in parts else 0):
                    b0 = g * BPG
                    nb = min(BPG, NBLK - b0)
                    w = nb * P
                    aggT = gpool.tile([D, GCOL], F32, tag="aggT",
                                      name=f"agg_{rep}_{layer}_{g}")
                    for bi in range(nb):
                        b = b0 + bi
                        ps = psA.tile([D, P], F32, tag="agg", name=f"ps_{layer}_{b}")
                        gts = meta.block_tiles[b]
                        for j, gt in enumerate(gts):
                            oh = ohpool.tile([P, P], BF16, tag="oh",
                                             name=f"oh_{layer}_{b}_{j}")
                            nc.vector.tensor_scalar(
                                out=oh, in0=iota_sb,
                                scalar1=dstf_sb[:, gt:gt + 1],
                                scalar2=None,
                                op0=mybir.AluOpType.is_equal,
                            )
                            mt, lt = tsrc[gt]
                            nc.tensor.matmul(
                                ps, lhsT=mt[:, lt, 0:D], rhs=oh,
                                start=(j == 0), stop=(j == len(gts) - 1),
                            )
                        # exact mean scaling: psum * (1/deg) broadcast table
                        nc.vector.tensor_tensor(
                            out=aggT[:, bi * P:(bi + 1) * P], in0=ps,
                            in1=invb_sb[:, b * P:(b + 1) * P],
                            op=mybir.AluOpType.mult,
                        )
                    if "dense" not in parts:
                        continue
                    zp = psZ.tile([D, GCOL], F32, tag="z", name=f"z_{layer}_{g}")
                    nc.tensor.matmul(zp[:, :w], lhsT=wl_sb, rhs=aggT[:, :w],
                                     start=True, stop=False)
                    nc.tensor.matmul(zp[:, :w], lhsT=wr_sb,
                                     rhs=own_sb[:, b0 * P:b0 * P + w],
                                     start=False, stop=True)
                    if layer == 0:
                        outT = hT_sb
                        nc.scalar.activation(out=hT_sb[:, b0 * P:b0 * P + w],
                                             in_=zp[:, :w], func=func,
                                             bias=bb_sb[:, 0:1], scale=1.0)
                    else:
                        outT = gpool.tile([D, GCOL], F32, tag="outT",
                                          name=f"oT_{rep}_{g}")
                        nc.scalar.activation(out=outT[:, :w], in_=zp[:, :w],
                                             func=func, bias=bb_sb[:, 0:1],
                                             scale=1.0)
                    if "store" not in parts:
                        continue
                    for bi in range(nb):
                        b = b0 + bi
                        tp = psT.tile([P, D], F32, tag="tr", name=f"tp_{layer}_{b}")
                        sl = (slice(b * P, b * P + P) if layer == 0
                              else slice(bi * P, bi * P + P))
                        nc.tensor.transpose(out=tp, in_=outT[:, sl],
                                            identity=ident_sb)
                        if layer == 0:
                            # bf16 padded node-major h rows
                            nc.scalar.copy(out=nodeh_sb[:, b * P:b * P + D],
                                           in_=tp)
                        else:
                            nc.scalar.copy(out=nodeo_sb[:, b * D:(b + 1) * D],
                                           in_=tp)

                if "store" in parts:
                    if layer == 0:
                        nc.sync.dma_start(
                            out=h_chunk.rearrange("(b p) f -> p b f", p=P),
                            in_=nodeh_sb.rearrange("p (b f) -> p b f", f=P),
                        )
                    else:
                        nc.sync.dma_start(
                            out=out_dr.ap().rearrange("(b p) f -> p b f", p=P),
                            in_=nodeo_sb.rearrange("p (b f) -> p b f", f=D),
                        )
                if layer == 0 and "collective" in parts:
                    if one_core:
                        nc.sync.dma_start(out=h_full[0:NP, :], in_=h_chunk)
                    else:
                        nc.gpsimd.collective_compute(
                            "AllGather",
                            mybir.AluOpType.bypass,
                            replica_groups=[list(range(NC))],
                            ins=[h_chunk.opt()],
                            outs=[h_full.opt()],
                        )

    nc.compile()
    return nc


def make_in_maps(meta, x, W_l1, b_l1, W_r1, W_l2, b_l2, W_r2):
    cfg = meta.cfg
    x = np.ascontiguousarray(np.asarray(x, dtype=np.float32))
    xp = np.zeros((cfg.N, P), BF)
    xp[:, :D] = x.astype(BF)
    iota = np.tile(np.arange(P, dtype=np.float32), (P, 1)).astype(BF)
    ident = np.eye(D, dtype=np.float32)
    common = {
        "xp": xp,
        "wl1t": np.ascontiguousarray(np.asarray(W_l1, np.float32).T),
        "wr1t": np.ascontiguousarray(np.asarray(W_r1, np.float32).T),
        "wl2t": np.ascontiguousarray(np.asarray(W_l2, np.float32).T),
        "wr2t": np.ascontiguousarray(np.asarray(W_r2, np.float32).T),
        "b1": np.asarray(b_l1, np.float32).reshape(D, 1).copy(),
        "b2": np.asarray(b_l2, np.float32).reshape(D, 1).copy(),
        "iota": iota,
        "ident": ident,
    }
    in_maps = []
    for k in range(cfg.n_cores):
        xo = x[k * cfg.n_own:(k + 1) * cfg.n_own]
        xoT = np.zeros((D, cfg.n_own_pad), np.float32)
        xoT[:, :cfg.n_own] = xo.T
        in_maps.append(dict(common, xoT=xoT, idx=meta.idx[k],
                            dstf=meta.dstf[k], invb=meta.invb[k]))
    return in_maps


_CACHE = {}
_LAST_RES = None


def kernel(x, edge_index, W_l1, b_l1, W_r1, W_l2, b_l2, W_r2):
    edge_index = np.asarray(edge_index)
    x = np.asarray(x)
    cfg = Cfg(x.shape[0])
    key = hash(edge_index.tobytes())
    if key in _CACHE:
        meta, nc = _CACHE[key]
    else:
        meta = preprocess(edge_index, cfg)
        nc = build_program(meta)
        _CACHE[key] = (meta, nc)
    in_maps = make_in_maps(meta, x, W_l1, b_l1, W_r1, W_l2, b_l2, W_r2)
    res = run_bass_kernel_spmd(nc, in_maps, core_ids=list(range(cfg.n_cores)))
    global _LAST_RES
    _LAST_RES = res
    out = np.concatenate(
        [res.results[k]["out"][:cfg.n_own] for k in range(cfg.n_cores)], axis=0
    )
    return out.astype(np.float32)



# revision 12
# speedup vs baseline: 2.3044x; 2.3044x over previous
"""2-layer GraphSAGE (mean aggregation) on 8 trn2 NeuronCores via Bass/Tile.

Strategy (matches the sharding hint):
  - Nodes are row-sharded across the 8 cores (6250 rows each); edges are
    partitioned by destination core.
  - Per core, edges are grouped by 128-node destination block.  Messages
    x[src] are fetched with InstDMAGatherAnt (edge-major tiles of 128) from a
    bf16 256B-padded copy of the features, and the segment-sum is computed as
    a one-hot matmul on the tensor engine (bf16, 1 cycle/row):
        aggT[64f, 128d] += msgs[128e, 64f].T @ onehot[128e, 128d]
    where onehot[e, d] = (d == dst_local[e]) is built on the vector engine
    from a broadcast iota with one tensor_scalar(is_equal) op per tile.
    The exact f32 1/deg scaling is applied at PSUM->SBUF copy time via a
    host-built [64, n] broadcast table (elementwise mult on DVE).
  - The 64x64 weights are replicated; the dense phase runs feature-major in
    f32 on rotating [64, 512] group buffers.
  - h = tanh(layer1) chunks (bf16-padded) are AllGathered between layers.
  - dma_gather indices are int16, so each gather call reads one of two row
    regions of the source, split at a core boundary so lo/hi membership is
    identical for the x-space and the padded h-space.
"""

import numpy as np
import ml_dtypes

import concourse.bacc as bacc
import concourse.mybir as mybir
import concourse.tile as tile
from concourse.bass import AP
from concourse.bass_utils import run_bass_kernel_spmd

P = 128
D = 64
F32 = mybir.dt.float32
BF16 = mybir.dt.bfloat16
I16 = mybir.dt.int16
BF = ml_dtypes.bfloat16


class Cfg:
    def __init__(self, N, n_cores=8, chunk=64, msgs_bufs=3, nqueues=4,
                 oh_bufs=2, single_packet=False):
        assert N % n_cores == 0
        self.N = N
        self.n_cores = n_cores
        self.n_own = N // n_cores
        self.nblk = -(-self.n_own // P)
        self.n_own_pad = self.nblk * P
        self.n_pad_all = self.n_own_pad * n_cores
        # lo/hi split at a core boundary so that edge region membership is
        # identical for x-space (N rows) and padded h-space (n_pad_all rows).
        c = n_cores // 2
        while self.N - c * self.n_own > 32768 or self.n_pad_all - c * self.n_own_pad > 32768:
            c += 1
        assert c * self.n_own <= 32768 and c * self.n_own_pad <= 32768
        self.split_core = c
        self.split = c * self.n_own
        self.split_pad = c * self.n_own_pad
        self.chunk = chunk
        self.msgs_bufs = msgs_bufs
        self.nqueues = nqueues
        self.oh_bufs = oh_bufs
        self.single_packet = single_packet


class Meta:
    pass


def _wrap16(v):
    """slot i -> [i % 16, i // 16] layout used by dma_gather idx tables."""
    assert v.shape[0] % 16 == 0
    return np.ascontiguousarray(v.reshape(-1, 16).T)


def preprocess(edge_index, cfg):
    """Partition/group edges; build per-core gather index + onehot tables."""
    src = np.asarray(edge_index[0], dtype=np.int64)
    dst = np.asarray(edge_index[1], dtype=np.int64)
    E = src.shape[0]
    NC, NBLK = cfg.n_cores, cfg.nblk

    cnt = np.bincount(dst, minlength=cfg.N).astype(np.float32)
    inv = (1.0 / np.maximum(cnt, 1.0)).astype(np.float32)

    core = dst // cfg.n_own
    dstl = dst - core * cfg.n_own
    blk = dstl // P
    inb = dstl - blk * P
    region = (src >= cfg.split).astype(np.int64)

    key = ((core * NBLK) + blk) * 2 + region
    ngroups = NC * NBLK * 2
    gcnt = np.bincount(key, minlength=ngroups).reshape(NC, NBLK, 2)
    # uniform (max over cores) tile counts per (block, region)
    TL = np.maximum(1, -(-gcnt[:, :, 0].max(axis=0) // P))
    TH = np.maximum(1, -(-gcnt[:, :, 1].max(axis=0) // P))
    lo_off = np.concatenate([[0], np.cumsum(TL)])
    hi_off = np.concatenate([[0], np.cumsum(TH)])
    TLT, THT = int(lo_off[-1]), int(hi_off[-1])
    T_ALL = TLT + THT

    # rank of each edge within its (core, blk, region) group
    order = np.argsort(key, kind="stable")
    gstart = np.concatenate([[0], np.cumsum(np.bincount(key, minlength=ngroups))])[:-1]
    rank = np.empty(E, dtype=np.int64)
    rank[order] = np.arange(E) - gstart[key[order]]

    # slot within region (tiles of 128)
    reg_base = np.where(region == 0, lo_off[blk], hi_off[blk])
    slot = reg_base * P + rank

    # h-space (padded) position of each source node
    pos = (src // cfg.n_own) * cfg.n_own_pad + (src % cfg.n_own)

    meta = Meta()
    meta.cfg = cfg
    meta.TL, meta.TH = TL, TH
    meta.TLT, meta.THT, meta.T_ALL = TLT, THT, T_ALL
    meta.block_tiles = [
        list(range(int(lo_off[b]), int(lo_off[b + 1])))
        + [TLT + t for t in range(int(hi_off[b]), int(hi_off[b + 1]))]
        for b in range(NBLK)
    ]

    # per-core tables
    meta.idx = []   # [128, 8*(TLT+THT)*2] int16 : l1lo | l1hi | l2lo | l2hi
    meta.dstf = []  # [128, T_ALL] f32
    meta.invb = []  # [64, n_own_pad] f32 : 1/deg broadcast down 64 partitions
    for k in range(NC):
        m = core == k
        sl = slot[m]
        rg = region[m]
        s_lo, s_hi = sl[rg == 0], sl[rg == 1]
        i1lo = np.zeros(TLT * P, np.int16)
        i1hi = np.zeros(THT * P, np.int16)
        i2lo = np.zeros(TLT * P, np.int16)
        i2hi = np.zeros(THT * P, np.int16)
        i1lo[s_lo] = src[m][rg == 0]
        i1hi[s_hi] = src[m][rg == 1] - cfg.split
        i2lo[s_lo] = pos[m][rg == 0]
        i2hi[s_hi] = pos[m][rg == 1] - cfg.split_pad
        w = np.concatenate(
            [_wrap16(a) for a in (i1lo, i1hi, i2lo, i2hi)], axis=1)
        # the gather ucode reads each Q7 core's idx stripe from its own
        # 16-partition group -> replicate 8x down the partition axis
        meta.idx.append(np.ascontiguousarray(np.tile(w, (8, 1))))

        df = np.full(T_ALL * P, -1.0, np.float32)
        gs = np.where(rg == 0, 0, TLT * P) + sl
        df[gs] = inb[m].astype(np.float32)
        meta.dstf.append(np.ascontiguousarray(df.reshape(T_ALL, P).T))

        iv = np.ones(cfg.n_own_pad, np.float32)
        iv[:cfg.n_own] = inv[k * cfg.n_own:(k + 1) * cfg.n_own]
        meta.invb.append(np.ascontiguousarray(np.tile(iv, (D, 1))))

    meta.idx_off = [0, TLT * 8, (TLT + THT) * 8, (2 * TLT + THT) * 8]

    # gather calls: (region, t0, ntiles, first_block), interleaved by the
    # first destination block each chunk serves.
    def chunks(T_total, offs):
        out = []
        t0 = 0
        while t0 < T_total:
            nt = min(cfg.chunk, T_total - t0)
            fb = int(np.searchsorted(offs, t0, side="right") - 1)
            out.append((t0, nt, fb))
            t0 += nt
        return out

    calls = [(0, t0, nt, fb) for (t0, nt, fb) in chunks(TLT, lo_off)]
    calls += [(1, t0, nt, fb) for (t0, nt, fb) in chunks(THT, hi_off)]
    calls.sort(key=lambda c: (c[3], c[0]))
    meta.calls = calls
    return meta


GCOL = 512  # dense-phase group width (one PSUM bank)


def build_program(meta, one_core=False,
                  parts=("gather", "agg", "dense", "store", "collective"),
                  reps=1):
    cfg = meta.cfg
    NC, NBLK = cfg.n_cores, cfg.nblk
    NP = cfg.n_own_pad
    BPG = GCOL // P  # blocks per dense group
    nc = bacc.Bacc(
        "TRN2", target_bir_lowering=False, debug=False,
        num_devices=1 if one_core else NC,
        num_swdge_queues=cfg.nqueues,
    )

    xp_dr = nc.dram_tensor("xp", [cfg.N, P], BF16, kind="ExternalInput")
    xoT_dr = nc.dram_tensor("xoT", [D, NP], F32, kind="ExternalInput")
    idx_dr = nc.dram_tensor("idx", list(meta.idx[0].shape), I16, kind="ExternalInput")
    dstf_dr = nc.dram_tensor("dstf", [P, meta.T_ALL], F32, kind="ExternalInput")
    invb_dr = nc.dram_tensor("invb", [D, NP], F32, kind="ExternalInput")
    wl1_dr = nc.dram_tensor("wl1t", [D, D], F32, kind="ExternalInput")
    wr1_dr = nc.dram_tensor("wr1t", [D, D], F32, kind="ExternalInput")
    wl2_dr = nc.dram_tensor("wl2t", [D, D], F32, kind="ExternalInput")
    wr2_dr = nc.dram_tensor("wr2t", [D, D], F32, kind="ExternalInput")
    b1_dr = nc.dram_tensor("b1", [D, 1], F32, kind="ExternalInput")
    b2_dr = nc.dram_tensor("b2", [D, 1], F32, kind="ExternalInput")
    iota_dr = nc.dram_tensor("iota", [P, P], BF16, kind="ExternalInput")
    id_dr = nc.dram_tensor("ident", [D, D], F32, kind="ExternalInput")
    out_dr = nc.dram_tensor("out", [NP, D], F32, kind="ExternalOutput")

    with tile.TileContext(nc) as tc:
        with (
            tc.tile_pool(name="const", bufs=1) as cpool,
            tc.tile_pool(name="big", bufs=1) as bpool,
            tc.tile_pool(name="msgs", bufs=cfg.msgs_bufs) as mpool,
            tc.tile_pool(name="idxp", bufs=4) as ipool,
            tc.tile_pool(name="ohp", bufs=cfg.oh_bufs) as ohpool,
            tc.tile_pool(name="grp", bufs=2) as gpool,
            tc.tile_pool(name="psA", bufs=4, space="PSUM") as psA,
            tc.tile_pool(name="psZ", bufs=2, space="PSUM") as psZ,
            tc.tile_pool(name="psT", bufs=2, space="PSUM") as psT,
            tc.tile_pool(name="dram", bufs=1, space="DRAM") as dpool,
        ):
            def load(pool, dr, shape, name, dt=F32, tag=""):
                t = pool.tile(shape, dt, name=name, tag=tag or name)
                nc.sync.dma_start(out=t, in_=dr.ap())
                return t

            iota_sb = load(cpool, iota_dr, [P, P], "iota_sb", dt=BF16)
            ident_sb = load(cpool, id_dr, [D, D], "ident_sb")
            wl1_sb = load(cpool, wl1_dr, [D, D], "wl1_sb")
            wr1_sb = load(cpool, wr1_dr, [D, D], "wr1_sb")
            wl2_sb = load(cpool, wl2_dr, [D, D], "wl2_sb")
            wr2_sb = load(cpool, wr2_dr, [D, D], "wr2_sb")
            b1_sb = load(cpool, b1_dr, [D, 1], "b1_sb")
            b2_sb = load(cpool, b2_dr, [D, 1], "b2_sb")
            dstf_sb = load(bpool, dstf_dr, [P, meta.T_ALL], "dstf_sb")
            invb_sb = load(bpool, invb_dr, [D, NP], "invb_sb")
            xoT_sb = load(bpool, xoT_dr, [D, NP], "xoT_sb")
            hT_sb = bpool.tile([D, NP], F32, name="hT_sb")
            nodeh_sb = bpool.tile([P, NBLK * P], BF16, name="nodeh_sb")
            nodeo_sb = bpool.tile([P, NBLK * D], F32, name="nodeo_sb")
            # zero the bf16 pad columns once (cols [b*128+64, b*128+128))
            nc.vector.memset(nodeh_sb, 0.0)

            for rep in range(reps):
              h_chunk = dpool.tile([NP, P], BF16, name=f"h_chunk_{rep}", tag=f"hc{rep}")
              h_full = dpool.tile([cfg.n_pad_all, P], BF16, name=f"h_full_{rep}",
                                  tag=f"hf{rep}", addr_space="Shared")
              for layer in range(2):
                if layer == 0:
                    src_lo = xp_dr.ap()[0:cfg.split, :]
                    src_hi = xp_dr.ap()[cfg.split:cfg.N, :]
                    off_lo, off_hi = meta.idx_off[0], meta.idx_off[1]
                else:
                    src_lo = h_full[0:cfg.split_pad, :]
                    src_hi = h_full[cfg.split_pad:cfg.n_pad_all, :]
                    off_lo, off_hi = meta.idx_off[2], meta.idx_off[3]

                # ---- gather messages (bf16, 256B rows) ----
                tsrc = {}
                ohsrc = {}
                for ci, (rg, t0, ntile, _fb) in enumerate(meta.calls):
                    mt = mpool.tile([P, cfg.chunk, P], BF16, tag="msgs",
                                    name=f"m_{layer}_{ci}")
                    if "gather" in parts:
                        it = ipool.tile([P, cfg.chunk * 8], I16, tag="idx",
                                        name=f"i_{layer}_{ci}")
                        cols = ntile * 8
                        coff = (off_lo if rg == 0 else off_hi) + t0 * 8
                        nc.sync.dma_start(out=it[:, :cols],
                                          in_=idx_dr.ap()[:, coff:coff + cols])
                        nc.gpsimd.dma_gather(
                            mt[:, :ntile, :],
                            src_lo if rg == 0 else src_hi,
                            it[:, :cols],
                            num_idxs=ntile * P,
                            num_idxs_reg=ntile * P,
                            elem_size=P,
                            single_packet=cfg.single_packet,
                            queue_num=ci % cfg.nqueues,
                        )
                    base = t0 if rg == 0 else meta.TLT + t0
                    # one broadcast-AP DVE op builds this whole chunk's onehots:
                    #   oh[e, t, d] = (iota[e, d] == dstf[e, base + t])
                    oht = ohpool.tile([P, cfg.chunk, P], BF16, tag="oh",
                                      name=f"oh_{layer}_{ci}")
                    if "agg" in parts:
                        io = iota_sb[:, :]
                        in0 = AP(io.tensor, io.offset,
                                 [io.ap[0], [0, ntile], [1, P]])
                        df = dstf_sb[:, base:base + ntile]
                        in1 = AP(df.tensor, df.offset,
                                 [df.ap[0], df.ap[1], [0, P]])
                        nc.vector.tensor_tensor(
                            out=oht[:, :ntile, :], in0=in0, in1=in1,
                            op=mybir.AluOpType.is_equal,
                        )
                    for j in range(ntile):
                        tsrc[base + j] = (mt, j)
                        ohsrc[base + j] = (oht, j)

                # ---- blocks: onehot matmul segment-sum + dense per group ----
                if layer == 0:
                    wl_sb, wr_sb, bb_sb = wl1_sb, wr1_sb, b1_sb
                    own_sb = xoT_sb
                    func = mybir.ActivationFunctionType.Tanh
                else:
                    wl_sb, wr_sb, bb_sb = wl2_sb, wr2_sb, b2_sb
                    own_sb = hT_sb
                    func = mybir.ActivationFunctionType.Identity

                ngrp = -(-NBLK // BPG)
                for g in range(ngrp if "agg" in parts else 0):
                    b0 = g * BPG
                    nb = min(BPG, NBLK - b0)
                    w = nb * P
                    aggT = gpool.tile([D, GCOL], F32, tag="aggT",
                                      name=f"agg_{rep}_{layer}_{g}")
                    for bi in range(nb):
                        b = b0 + bi
                        ps = psA.tile([D, P], F32, tag="agg", name=f"ps_{layer}_{b}")
                        gts = meta.block_tiles[b]
                        for j, gt in enumerate(gts):
                            oht, lo = ohsrc[gt]
                            mt, lt = tsrc[gt]
                            nc.tensor.matmul(
                                ps, lhsT=mt[:, lt, 0:D], rhs=oht[:, lo, :],
                                start=(j == 0), stop=(j == len(gts) - 1),
                            )
                        # exact mean scaling: psum * (1/deg) broadcast table
                        nc.vector.tensor_tensor(
                            out=aggT[:, bi * P:(bi + 1) * P], in0=ps,
                            in1=invb_sb[:, b * P:(b + 1) * P],
                            op=mybir.AluOpType.mult,
                        )
                    if "dense" not in parts:
                        continue
                    zp = psZ.tile([D, GCOL], F32, tag="z", name=f"z_{layer}_{g}")
                    nc.tensor.matmul(zp[:, :w], lhsT=wl_sb, rhs=aggT[:, :w],
                                     start=True, stop=False)
                    nc.tensor.matmul(zp[:, :w], lhsT=wr_sb,
                                     rhs=own_sb[:, b0 * P:b0 * P + w],
                                     start=False, stop=True)
                    if layer == 0:
                        outT = hT_sb
                        nc.scalar.activation(out=hT_sb[:, b0 * P:b0 * P + w],
                                             in_=zp[:, :w], func=func,
                                             bias=bb_sb[:, 0:1], scale=1.0)
                    else:
                        outT = gpool.tile([D, GCOL], F32, tag="outT",
                                          name=f"oT_{rep}_{g}")
                        nc.scalar.activation(out=outT[:, :w], in_=zp[:, :w],
                                             func=func, bias=bb_sb[:, 0:1],
                                             scale=1.0)
                    if "store" not in parts:
                        continue
                    for bi in range(nb):
                        b = b0 + bi
                        tp = psT.tile([P, D], F32, tag="tr", name=f"tp_{layer}_{b}")
                        sl = (slice(b * P, b * P + P) if layer == 0
                              else slice(bi * P, bi * P + P))
                        nc.tensor.transpose(out=tp, in_=outT[:, sl],
                                            identity=ident_sb)
                        if layer == 0:
                            # bf16 padded node-major h rows
                            nc.scalar.copy(out=nodeh_sb[:, b * P:b * P + D],
                                           in_=tp)
                        else:
                            nc.scalar.copy(out=nodeo_sb[:, b * D:(b + 1) * D],
                                           in_=tp)

                if "store" in parts:
                    if layer == 0:
                        nc.sync.dma_start(
                            out=h_chunk.rearrange("(b p) f -> p b f", p=P),
                            in_=nodeh_sb.rearrange("p (b f) -> p b f", f=P),
                        )
                    else:
                        nc.sync.dma_start(
                            out=out_dr.ap().rearrange("(b p) f -> p b f", p=P),
                            in_=nodeo_sb.rearrange("p (b f) -> p b f", f=D),
                        )
                if layer == 0 and "collective" in parts:
                    if one_core:
                        nc.sync.dma_start(out=h_full[0:NP, :], in_=h_chunk)
                    else:
                        nc.gpsimd.collective_compute(
                            "AllGather",
                            mybir.AluOpType.bypass,
                            replica_groups=[list(range(NC))],
                            ins=[h_chunk.opt()],
                            outs=[h_full.opt()],
                        )

    nc.compile()
    return nc


def make_in_maps(meta, x, W_l1, b_l1, W_r1, W_l2, b_l2, W_r2):
    cfg = meta.cfg
    x = np.ascontiguousarray(np.asarray(x, dtype=np.float32))
    xp = np.zeros((cfg.N, P), BF)
    xp[:, :D] = x.astype(BF)
    iota = np.tile(np.arange(P, dtype=np.float32), (P, 1)).astype(BF)
    ident = np.eye(D, dtype=np.float32)
    common = {
        "xp": xp,
        "wl1t": np.ascontiguousarray(np.asarray(W_l1, np.float32).T),
        "wr1t": np.ascontiguousarray(np.asarray(W_r1, np.float32).T),
        "wl2t": np.ascontiguousarray(np.asarray(W_l2, np.float32).T),
        "wr2t": np.ascontiguousarray(np.asarray(W_r2, np.float32).T),
        "b1": np.asarray(b_l1, np.float32).reshape(D, 1).copy(),
        "b2": np.asarray(b_l2, np.float32).reshape(D, 1).copy(),
        "iota": iota,
        "ident": ident,
    }
    in_maps = []
    for k in range(cfg.n_cores):
        xo = x[k * cfg.n_own:(k + 1) * cfg.n_own]
        xoT = np.zeros((D, cfg.n_own_pad), np.float32)
        xoT[:, :cfg.n_own] = xo.T
        in_maps.append(dict(common, xoT=xoT, idx=meta.idx[k],
                            dstf=meta.dstf[k], invb=meta.invb[k]))
    return in_maps


_CACHE = {}
_LAST_RES = None


def kernel(x, edge_index, W_l1, b_l1, W_r1, W_l2, b_l2, W_r2):
    edge_index = np.asarray(edge_index)
    x = np.asarray(x)
    cfg = Cfg(x.shape[0])
    key = hash(edge_index.tobytes())
    if key in _CACHE:
        meta, nc = _CACHE[key]
    else:
        meta = preprocess(edge_index, cfg)
        nc = build_program(meta)
        _CACHE[key] = (meta, nc)
    in_maps = make_in_maps(meta, x, W_l1, b_l1, W_r1, W_l2, b_l2, W_r2)
    res = run_bass_kernel_spmd(nc, in_maps, core_ids=list(range(cfg.n_cores)))
    global _LAST_RES
    _LAST_RES = res
    out = np.concatenate(
        [res.results[k]["out"][:cfg.n_own] for k in range(cfg.n_cores)], axis=0
    )
    return out.astype(np.float32)

